# revision 33
# baseline (speedup 1.0000x reference)
"""Trainium2 Bass kernel for nn_CorrBlockSingleScale (RAFT single-scale
correlation lookup), distributed over 8 NeuronCores.

  fmap1, fmap2: [1, 256, 64, 96] f32;  coords: [1, 2, 64, 96] f32; radius=4
  corr = einsum('bcm,bcn->bmn', f1, f2) / 16        -> [6144, 64, 96]
  out[q, i, j] = bilinear(corr[q], (cx_q + d_i, cy_q + d_j)),  d in -4..4
  output [1, 81, 64, 96] f32.

Structure exploited: the 9x9 sample offsets are integers, so all 81 samples
of a query share one fractional pair (fx, fy) -- the output is a separable
2x2-tap blend of a 10x10 patch of corr[q] anchored at
(floor(cx)-4, floor(cy)-4).

Distribution / tiling (no collectives): queries sorted by floor(cy),
chopped into 8 cores of 768; within a core re-sorted by floor(cx) and
chopped into 6 tiles of 128, so each tile's correlation band is only
BXF x SROWS cells (~30 x 18) of the 64x96 target plane.  The host ships
per-(core,tile) pre-sliced bf16 bands cut from a zero-padded plane
(x in [-5,101), y rows [ys, ys+SROWS)), so out-of-plane bilinear taps
read stored zeros -- no masks, and no cross-core anchor unification.

Per core and iteration (35 device instructions total):
  1. per tile: one accumulating bf16 matmul pair (K=256 = 2x128) into one
     PSUM bank (band <= 512 elems), cast to bf16 into a slice of one big
     corr tile (ACT/DVE alternating).
  2. ONE scratch write DMA [128, 6*BXF*SROWS] -> DRAM.
  3. per tile: indirect-DMA gather of each query's contiguous 172-element
     window (the 10x10 patch in the x-major band) into a slice of one
     patch tile.
  4. separable bilinear blend for ALL tiles at once: 6 tensor_tensor ops
     with per-(partition,tile) weights broadcast along the patch axes
     (stride-0 APs).  bf16 output rows, one DMA.
Host post-pass upcasts and inverse-permutes rows to reference layout.
"""

import contextlib

import numpy as np
import ml_dtypes

import concourse.bass as bass
import concourse.bacc as bacc
import concourse.mybir as mybir
import concourse.tile as tile
from concourse import bass_utils
from concourse.bass import broadcast_tensor_aps

BF16NP = ml_dtypes.bfloat16
F32 = mybir.dt.float32
I32 = mybir.dt.int32
BF16 = mybir.dt.bfloat16

B, C, H, W = 1, 256, 64, 96
R = 4
K = 2 * R + 1          # 9
PK = K + 1             # 10 (patch side)
NQ = H * W             # 6144
NCORES = 8
QPC = NQ // NCORES     # 768
P = 128
NT = QPC // P          # 6 tiles per core
KH = 2                 # K halves (256 = 2 x 128)
PADX = 5               # padded x coords [-5, 101)
PADY = 5               # padded y coords [-5, 69)
WPAD = W + 2 * PADX    # 106
NM = 5                 # meta cols: idx, wy0, wy1, wx0/16, wx1/16


# --------------------------------------------------------------------------
# host-side preprocessing
# --------------------------------------------------------------------------

def host_preprocess(fmap1, fmap2, coords):
    """Returns (in_maps, order, geom) with geom = (SROWS, BXF)."""
    f1 = np.asarray(fmap1, np.float32).reshape(C, NQ)
    f2 = np.asarray(fmap2, np.float32).reshape(C, H, W)
    cx_all = np.asarray(coords, np.float32)[0, 0].reshape(NQ)
    cy_all = np.asarray(coords, np.float32)[0, 1].reshape(NQ)
    ix_all = np.floor(cx_all).astype(np.int64)
    iy_all = np.floor(cy_all).astype(np.int64)

    yorder = np.lexsort((np.arange(NQ), ix_all, iy_all))
    order = np.empty(NQ, np.int64)
    for c in range(NCORES):
        qs = yorder[c * QPC:(c + 1) * QPC]
        sub = np.lexsort((np.arange(QPC), iy_all[qs], ix_all[qs]))
        order[c * QPC:(c + 1) * QPC] = qs[sub]

    plane = np.zeros((C, H + 2 * PADY, WPAD), np.float32)
    plane[:, PADY:PADY + H, PADX:PADX + W] = f2
    plane = plane.astype(BF16NP)

    cores = []
    for c in range(NCORES):
        qs = order[c * QPC:(c + 1) * QPC]
        jx = ix_all[qs]
        jy = iy_all[qs]
        fx = (cx_all[qs] - jx).astype(np.float32)
        fy = (cy_all[qs] - jy).astype(np.float32)
        cores.append(dict(qs=qs, jx=jx, jy=jy, fx=fx, fy=fy))

    SROWS = max(int(c["jy"].max() - c["jy"].min()) + PK for c in cores)
    for c in cores:
        ys = int(c["jy"].min()) - R
        ys = max(min(ys, H + PADY - SROWS), -PADY)
        c["ys"] = ys
        assert ys <= c["jy"].min() - R
        assert ys + SROWS >= c["jy"].max() + R + 2

    # per-(core,tile) x anchors; uniform band width = max extent
    BXF = 0
    for c in cores:
        axs = []
        for t in range(NT):
            sel = slice(t * P, (t + 1) * P)
            lo = int(c["jx"][sel].min()) - R
            hi = int(c["jx"][sel].max()) + R + 1
            axs.append((lo, hi - lo + 1))
            BXF = max(BXF, hi - lo + 1)
        c["axs"] = axs
    for c in cores:
        c["ax"] = [max(min(lo, W + PADX - BXF), -PADX)
                   for lo, _ in c["axs"]]
        for t in range(NT):
            sel = slice(t * P, (t + 1) * P)
            assert c["ax"][t] <= c["jx"][sel].min() - R
            assert c["ax"][t] + BXF >= c["jx"][sel].max() + R + 2

    BXFS = BXF * SROWS
    f1b = f1.astype(BF16NP)
    in_maps = []
    for c in cores:
        qs, jx, jy, ys = c["qs"], c["jx"], c["jy"], c["ys"]
        f1s = np.ascontiguousarray(f1b[:, qs].reshape(KH, P, QPC))

        bands = np.empty((NT, KH, P, BXFS), BF16NP)
        for t in range(NT):
            colsel = plane[:, ys + PADY: ys + PADY + SROWS,
                           c["ax"][t] + PADX: c["ax"][t] + PADX + BXF]
            bands[t] = colsel.transpose(0, 2, 1).reshape(KH, P, BXFS)

        meta = np.zeros((QPC, NM), np.int32)
        for t in range(NT):
            sel = slice(t * P, (t + 1) * P)
            dx = jx[sel] - R - c["ax"][t]
            dy = jy[sel] - R - ys
            idx = np.arange(P) * BXFS + dx * SROWS + dy
            meta[sel, 0] = idx.astype(np.int32)
            meta[sel, 1] = (1.0 - c["fy"][sel]).astype(np.float32) \
                .view(np.int32)
            meta[sel, 2] = c["fy"][sel].astype(np.float32).view(np.int32)
            meta[sel, 3] = ((1.0 - c["fx"][sel]) / 16.0) \
                .astype(np.float32).view(np.int32)
            meta[sel, 4] = (c["fx"][sel] / 16.0).astype(np.float32) \
                .view(np.int32)

        in_maps.append({"f1s": f1s, "f2s": np.ascontiguousarray(bands),
                        "meta": meta})

    geom = (SROWS, BXF)
    return in_maps, order, geom


def assemble_output(results, order):
    rows = np.concatenate(
        [results[c]["out"].astype(np.float32) for c in range(NCORES)], axis=0)
    full = np.empty((K * K, NQ), np.float32)
    full[:, order] = rows.T
    return full.reshape(1, K * K, H, W)


# --------------------------------------------------------------------------
# device program
# --------------------------------------------------------------------------

def _body(tc, nc, aps, scr, geom, pools, u, ab=frozenset()):
    SROWS, BXF = geom
    BXFS = BXF * SROWS
    GW = PK * SROWS             # per-tile patch segment (gather fills 172)
    win = (PK - 1) * SROWS + PK
    const, corr_pool, psum_pool, small = pools

    f1sb = const.tile([P, KH * QPC], BF16, tag="f1sb")
    nc.sync.dma_start(
        f1sb[:].rearrange("p (k m) -> p k m", k=KH),
        aps["f1s"].rearrange("k p m -> p k m"))
    f2sb = const.tile([P, NT * KH * BXFS], BF16, tag="f2sb")
    nc.sync.dma_start(
        f2sb[:].rearrange("p (t k m) -> p t k m", t=NT, k=KH),
        aps["f2s"].rearrange("t k p m -> p t k m"))
    metab = const.tile([P, NT * NM], I32, tag="metab")
    nc.sync.dma_start(
        metab[:].rearrange("p (t a) -> p t a", a=NM),
        aps["meta"].rearrange("(t p) a -> p t a", p=P))
    m3 = metab[:].rearrange("p (t a) -> p t a", a=NM)

    ptall = small.tile([P, NT * GW], BF16, tag="ptall")
    if "gather" in ab or "mm" in ab:
        nc.vector.memset(ptall[:], 0.0)
    nchunk = -(-BXFS // 512)
    cw = -(-BXF // nchunk) * SROWS
    for t in range(NT if "mm" not in ab else 0):
        corr_sb = corr_pool.tile([P, BXFS], BF16, tag="corr")
        x0 = 0
        ci = 0
        while x0 < BXFS:
            w = min(cw, BXFS - x0)
            ps = psum_pool.tile([P, 512], F32, space="PSUM", tag="ps")
            for kh in range(KH):
                lhsT = f1sb[:, kh * QPC + t * P: kh * QPC + (t + 1) * P]
                rhs = f2sb[:, (t * KH + kh) * BXFS + x0:
                           (t * KH + kh) * BXFS + x0 + w]
                nc.tensor.matmul(ps[:, :w], lhsT=lhsT, rhs=rhs,
                                 start=(kh == 0), stop=(kh == KH - 1))
            dst = corr_sb[:, x0: x0 + w]
            if (t + ci) % 2 == 0:
                nc.scalar.copy(dst, ps[:, :w])
            else:
                nc.vector.tensor_copy(dst, ps[:, :w])
            x0 += w
            ci += 1

        if "write" not in ab:
            sdst = scr[t].ap()[0: P * BXFS].rearrange("(p f) -> p f", p=P)
            nc.sync.dma_start(sdst, corr_sb[:])
        if "gather" not in ab:
            src = scr[t].ap().rearrange("(n o) -> n o", o=1)
            nc.gpsimd.indirect_dma_start(
                out=ptall[:, t * GW: t * GW + win], out_offset=None,
                in_=src,
                in_offset=bass.IndirectOffsetOnAxis(
                    ap=metab[:, NM * t: NM * t + 1], axis=0))

    if "blend" in ab:
        otall = small.tile([P, NT * K * K], BF16, tag="otall")
        nc.vector.memset(otall[:], 0.0)
        nc.sync.dma_start(
            aps["out"].rearrange("(t p) k -> p t k", p=P),
            otall[:].rearrange("p (t k) -> p t k", k=K * K))
        return

    # broadcast per-(partition, tile) weights over the patch axes
    pt4 = ptall[:].rearrange("p (t b r) -> p t b r", b=PK, r=SROWS)

    def wb(a, like):
        wv = m3[:, :, 1 + a: 2 + a].bitcast(F32) \
            .rearrange("p t (x y) -> p t x y", x=1)
        return broadcast_tensor_aps(wv, like)[0]

    t1 = small.tile([P, NT * PK * K], F32, tag="t1")
    t14 = t1[:].rearrange("p (t a b) -> p t a b", t=NT, b=K)
    in_hi = pt4[:, :, :, 1:PK]
    nc.vector.tensor_tensor(t14, in_hi, wb(1, in_hi),
                            op=mybir.AluOpType.mult)
    u1 = small.tile([P, NT * PK * K], F32, tag="u1")
    u14 = u1[:].rearrange("p (t a b) -> p t a b", t=NT, b=K)
    in_lo = pt4[:, :, :, 0:K]
    nc.vector.tensor_tensor(u14, in_lo, wb(0, in_lo),
                            op=mybir.AluOpType.mult)
    cm = small.tile([P, NT * PK * K], F32, tag="cm")
    nc.vector.tensor_tensor(cm[:], t1[:], u1[:], op=mybir.AluOpType.add)
    cm4 = cm[:].rearrange("p (t a b) -> p t a b", t=NT, b=K)

    t2 = small.tile([P, NT * K * K], F32, tag="t2")
    t24 = t2[:].rearrange("p (t a b) -> p t a b", t=NT, b=K)
    cm_hi = cm4[:, :, 1:PK, :]
    nc.vector.tensor_tensor(t24, cm_hi, wb(3, cm_hi),
                            op=mybir.AluOpType.mult)
    u2 = small.tile([P, NT * K * K], F32, tag="u2")
    u24 = u2[:].rearrange("p (t a b) -> p t a b", t=NT, b=K)
    cm_lo = cm4[:, :, 0:K, :]
    nc.vector.tensor_tensor(u24, cm_lo, wb(2, cm_lo),
                            op=mybir.AluOpType.mult)
    otall = small.tile([P, NT * K * K], BF16, tag="otall")
    nc.vector.tensor_tensor(otall[:], t2[:], u2[:], op=mybir.AluOpType.add)

    nc.sync.dma_start(
        aps["out"].rearrange("(t p) k -> p t k", p=P),
        otall[:].rearrange("p (t k) -> p t k", k=K * K))


def build_program(geom, rep=1, unroll=1, ab=frozenset()):
    """rep = number of For_i iterations; each runs `unroll` bodies."""
    SROWS, BXF = geom
    nc = bacc.Bacc("TRN2", target_bir_lowering=False, debug=False,
                   num_devices=NCORES)
    aps = {}
    aps["f1s"] = nc.dram_tensor("f1s", [KH, P, QPC], BF16,
                                kind="ExternalInput").ap()
    aps["f2s"] = nc.dram_tensor("f2s", [NT, KH, P, BXF * SROWS], BF16,
                                kind="ExternalInput").ap()
    aps["meta"] = nc.dram_tensor("meta", [QPC, NM], I32,
                                 kind="ExternalInput").ap()
    aps["out"] = nc.dram_tensor("out", [QPC, K * K], BF16,
                                kind="ExternalOutput").ap()
    scr = [[nc.dram_tensor(f"scr{i}_{t}", [P * BXF * SROWS], BF16)
            for t in range(NT)] for i in range(min(2, unroll))]

    with tile.TileContext(nc) as tc:
        ctx = contextlib.ExitStack()
        with ctx:
            const = ctx.enter_context(tc.tile_pool(name="const", bufs=3))
            corr_pool = ctx.enter_context(tc.tile_pool(name="corr", bufs=3))
            psum_pool = ctx.enter_context(
                tc.tile_pool(name="ps", bufs=8, space="PSUM"))
            small = ctx.enter_context(tc.tile_pool(name="small", bufs=3))
            pools = (const, corr_pool, psum_pool, small)
            if rep == 1:
                for u in range(unroll):
                    _body(tc, nc, aps, scr[u % len(scr)], geom, pools, u, ab)
            else:
                with tc.For_i(0, rep):
                    for u in range(unroll):
                        _body(tc, nc, aps, scr[u % len(scr)], geom, pools,
                              u, ab)
    nc.compile()
    return nc


_PROGRAMS = {}


def kernel(fmap1, fmap2, coords, radius):
    assert int(radius) == R, f"kernel hardcodes radius=4, got {radius}"
    in_maps, order, geom = host_preprocess(fmap1, fmap2, coords)
    nc = _PROGRAMS.get(geom)
    if nc is None:
        nc = _PROGRAMS[geom] = build_program(geom)
    last_err = None
    for _ in range(3):  # the remote compile hook occasionally flakes
        try:
            res = bass_utils.run_bass_kernel_spmd(
                nc, in_maps, core_ids=list(range(NCORES)))
            return assemble_output(res.results, order)
        except Exception as e:  # noqa: BLE001
            last_err = e
    raise last_err


# revision 34
# speedup vs baseline: 1.1198x; 1.1198x over previous
"""Trainium2 Bass kernel for nn_CorrBlockSingleScale (RAFT single-scale
correlation lookup), distributed over 8 NeuronCores.

  fmap1, fmap2: [1, 256, 64, 96] f32;  coords: [1, 2, 64, 96] f32; radius=4
  corr = einsum('bcm,bcn->bmn', f1, f2) / 16        -> [6144, 64, 96]
  out[q, i, j] = bilinear(corr[q], (cx_q + d_i, cy_q + d_j)),  d in -4..4
  output [1, 81, 64, 96] f32.

Structure exploited: the 9x9 sample offsets are integers, so all 81 samples
of a query share one fractional pair (fx, fy) -- the output is a separable
2x2-tap blend of a 10x10 patch of corr[q] anchored at
(floor(cx)-4, floor(cy)-4).

Distribution / tiling (no collectives): queries sorted by floor(cy),
chopped into 8 cores of 768; within a core re-sorted by floor(cx) and
chopped into 6 tiles of 128, so each tile's correlation band is only
BXF x SROWS cells (~30 x 18) of the 64x96 target plane.  The host ships
per-(core,tile) pre-sliced bf16 bands cut from a zero-padded plane
(x in [-5,101), y rows [ys, ys+SROWS)), so out-of-plane bilinear taps
read stored zeros -- no masks, and no cross-core anchor unification.

Per core and iteration (35 device instructions total):
  1. per tile: one accumulating bf16 matmul pair (K=256 = 2x128) into one
     PSUM bank (band <= 512 elems), cast to bf16 into a slice of one big
     corr tile (ACT/DVE alternating).
  2. ONE scratch write DMA [128, 6*BXF*SROWS] -> DRAM.
  3. per tile: indirect-DMA gather of each query's contiguous 172-element
     window (the 10x10 patch in the x-major band) into a slice of one
     patch tile.
  4. separable bilinear blend for ALL tiles at once: 6 tensor_tensor ops
     with per-(partition,tile) weights broadcast along the patch axes
     (stride-0 APs).  bf16 output rows, one DMA.
Host post-pass upcasts and inverse-permutes rows to reference layout.
"""

import contextlib

import numpy as np
import ml_dtypes

import concourse.bass as bass
import concourse.bacc as bacc
import concourse.mybir as mybir
import concourse.tile as tile
from concourse import bass_utils
from concourse.bass import broadcast_tensor_aps

BF16NP = ml_dtypes.bfloat16
F32 = mybir.dt.float32
I32 = mybir.dt.int32
BF16 = mybir.dt.bfloat16

B, C, H, W = 1, 256, 64, 96
R = 4
K = 2 * R + 1          # 9
PK = K + 1             # 10 (patch side)
NQ = H * W             # 6144
NCORES = 8
QPC = NQ // NCORES     # 768
P = 128
NT = QPC // P          # 6 tiles per core
KH = 2                 # K halves (256 = 2 x 128)
PADX = 5               # padded x coords [-5, 101)
PADY = 5               # padded y coords [-5, 69)
WPAD = W + 2 * PADX    # 106
NM = 5                 # meta cols: idx, wy0, wy1, wx0/16, wx1/16


# --------------------------------------------------------------------------
# host-side preprocessing
# --------------------------------------------------------------------------

def host_preprocess(fmap1, fmap2, coords):
    """Returns (in_maps, order, geom) with geom = (SROWS, BXF)."""
    f1 = np.asarray(fmap1, np.float32).reshape(C, NQ)
    f2 = np.asarray(fmap2, np.float32).reshape(C, H, W)
    cx_all = np.asarray(coords, np.float32)[0, 0].reshape(NQ)
    cy_all = np.asarray(coords, np.float32)[0, 1].reshape(NQ)
    ix_all = np.floor(cx_all).astype(np.int64)
    iy_all = np.floor(cy_all).astype(np.int64)

    yorder = np.lexsort((np.arange(NQ), ix_all, iy_all))
    order = np.empty(NQ, np.int64)
    for c in range(NCORES):
        qs = yorder[c * QPC:(c + 1) * QPC]
        sub = np.lexsort((np.arange(QPC), iy_all[qs], ix_all[qs]))
        order[c * QPC:(c + 1) * QPC] = qs[sub]

    plane = np.zeros((C, H + 2 * PADY, WPAD), np.float32)
    plane[:, PADY:PADY + H, PADX:PADX + W] = f2
    plane = plane.astype(BF16NP)

    cores = []
    for c in range(NCORES):
        qs = order[c * QPC:(c + 1) * QPC]
        jx = ix_all[qs]
        jy = iy_all[qs]
        fx = (cx_all[qs] - jx).astype(np.float32)
        fy = (cy_all[qs] - jy).astype(np.float32)
        cores.append(dict(qs=qs, jx=jx, jy=jy, fx=fx, fy=fy))

    SROWS = max(int(c["jy"].max() - c["jy"].min()) + PK for c in cores)
    for c in cores:
        ys = int(c["jy"].min()) - R
        ys = max(min(ys, H + PADY - SROWS), -PADY)
        c["ys"] = ys
        assert ys <= c["jy"].min() - R
        assert ys + SROWS >= c["jy"].max() + R + 2

    # per-(core,tile) x anchors; uniform band width = max extent
    BXF = 0
    for c in cores:
        axs = []
        for t in range(NT):
            sel = slice(t * P, (t + 1) * P)
            lo = int(c["jx"][sel].min()) - R
            hi = int(c["jx"][sel].max()) + R + 1
            axs.append((lo, hi - lo + 1))
            BXF = max(BXF, hi - lo + 1)
        c["axs"] = axs
    for c in cores:
        c["ax"] = [max(min(lo, W + PADX - BXF), -PADX)
                   for lo, _ in c["axs"]]
        for t in range(NT):
            sel = slice(t * P, (t + 1) * P)
            assert c["ax"][t] <= c["jx"][sel].min() - R
            assert c["ax"][t] + BXF >= c["jx"][sel].max() + R + 2

    BXFS = BXF * SROWS
    f1b = f1.astype(BF16NP)
    in_maps = []
    for c in cores:
        qs, jx, jy, ys = c["qs"], c["jx"], c["jy"], c["ys"]
        f1s = np.ascontiguousarray(f1b[:, qs].reshape(KH, P, QPC))

        bands = np.empty((NT, KH, P, BXFS), BF16NP)
        for t in range(NT):
            colsel = plane[:, ys + PADY: ys + PADY + SROWS,
                           c["ax"][t] + PADX: c["ax"][t] + PADX + BXF]
            bands[t] = colsel.transpose(0, 2, 1).reshape(KH, P, BXFS)

        meta = np.zeros((QPC, NM), np.int32)
        for t in range(NT):
            sel = slice(t * P, (t + 1) * P)
            dx = jx[sel] - R - c["ax"][t]
            dy = jy[sel] - R - ys
            idx = np.arange(P) * BXFS + dx * SROWS + dy
            meta[sel, 0] = idx.astype(np.int32)
            meta[sel, 1] = (1.0 - c["fy"][sel]).astype(np.float32) \
                .view(np.int32)
            meta[sel, 2] = c["fy"][sel].astype(np.float32).view(np.int32)
            meta[sel, 3] = ((1.0 - c["fx"][sel]) / 16.0) \
                .astype(np.float32).view(np.int32)
            meta[sel, 4] = (c["fx"][sel] / 16.0).astype(np.float32) \
                .view(np.int32)

        in_maps.append({"f1s": f1s, "f2s": np.ascontiguousarray(bands),
                        "meta": meta})

    geom = (SROWS, BXF)
    return in_maps, order, geom


def assemble_output(results, order):
    rows = np.concatenate(
        [results[c]["out"].astype(np.float32) for c in range(NCORES)], axis=0)
    full = np.empty((K * K, NQ), np.float32)
    full[:, order] = rows.T
    return full.reshape(1, K * K, H, W)


# --------------------------------------------------------------------------
# device program
# --------------------------------------------------------------------------

def _body(tc, nc, aps, scr, geom, pools, u, ab=frozenset()):
    SROWS, BXF = geom
    BXFS = BXF * SROWS
    GW = PK * SROWS             # per-tile patch segment (gather fills 172)
    win = (PK - 1) * SROWS + PK
    const, corr_pool, psum_pool, small = pools

    f1sb = const.tile([P, KH * QPC], BF16, tag="f1sb")
    nc.sync.dma_start(
        f1sb[:].rearrange("p (k m) -> p k m", k=KH),
        aps["f1s"].rearrange("k p m -> p k m"))
    # two half-loads so tiles 0-2 matmuls start while tiles 3-5 still land
    f2sb = const.tile([P, NT * KH * BXFS], BF16, tag="f2sb")
    f2v4 = f2sb[:].rearrange("p (t k m) -> p t k m", t=NT, k=KH)
    f2d4 = aps["f2s"].rearrange("t k p m -> p t k m")
    nc.sync.dma_start(f2v4[:, 0:NT // 2], f2d4[:, 0:NT // 2])
    nc.sync.dma_start(f2v4[:, NT // 2:], f2d4[:, NT // 2:])
    metab = const.tile([P, NT * NM], I32, tag="metab")
    nc.sync.dma_start(
        metab[:].rearrange("p (t a) -> p t a", a=NM),
        aps["meta"].rearrange("(t p) a -> p t a", p=P))
    m3 = metab[:].rearrange("p (t a) -> p t a", a=NM)

    ptall = small.tile([P, NT * GW], BF16, tag="ptall")
    if "gather" in ab or "mm" in ab:
        nc.vector.memset(ptall[:], 0.0)
    nchunk = -(-BXFS // 512)
    cw = -(-BXF // nchunk) * SROWS
    for t in range(NT if "mm" not in ab else 0):
        corr_sb = corr_pool.tile([P, BXFS], BF16, tag="corr")
        x0 = 0
        ci = 0
        while x0 < BXFS:
            w = min(cw, BXFS - x0)
            ps = psum_pool.tile([P, 512], F32, space="PSUM", tag="ps")
            for kh in range(KH):
                lhsT = f1sb[:, kh * QPC + t * P: kh * QPC + (t + 1) * P]
                rhs = f2sb[:, (t * KH + kh) * BXFS + x0:
                           (t * KH + kh) * BXFS + x0 + w]
                nc.tensor.matmul(ps[:, :w], lhsT=lhsT, rhs=rhs,
                                 start=(kh == 0), stop=(kh == KH - 1))
            dst = corr_sb[:, x0: x0 + w]
            if (t + ci) % 2 == 0:
                nc.scalar.copy(dst, ps[:, :w])
            else:
                nc.vector.tensor_copy(dst, ps[:, :w])
            x0 += w
            ci += 1

        if "write" not in ab:
            sdst = scr[t].ap()[0: P * BXFS].rearrange("(p f) -> p f", p=P)
            nc.sync.dma_start(sdst, corr_sb[:])
        if "gather" not in ab:
            src = scr[t].ap().rearrange("(n o) -> n o", o=1)
            nc.gpsimd.indirect_dma_start(
                out=ptall[:, t * GW: t * GW + win], out_offset=None,
                in_=src,
                in_offset=bass.IndirectOffsetOnAxis(
                    ap=metab[:, NM * t: NM * t + 1], axis=0))

    if "blend" in ab:
        otall = small.tile([P, NT * K * K], BF16, tag="otall")
        nc.vector.memset(otall[:], 0.0)
        nc.sync.dma_start(
            aps["out"].rearrange("(t p) k -> p t k", p=P),
            otall[:].rearrange("p (t k) -> p t k", k=K * K))
        return

    # broadcast per-(partition, tile) weights over the patch axes
    pt4 = ptall[:].rearrange("p (t b r) -> p t b r", b=PK, r=SROWS)

    def wb(a, like):
        wv = m3[:, :, 1 + a: 2 + a].bitcast(F32) \
            .rearrange("p t (x y) -> p t x y", x=1)
        return broadcast_tensor_aps(wv, like)[0]

    t1 = small.tile([P, NT * PK * K], F32, tag="t1")
    t14 = t1[:].rearrange("p (t a b) -> p t a b", t=NT, b=K)
    in_hi = pt4[:, :, :, 1:PK]
    nc.vector.tensor_tensor(t14, in_hi, wb(1, in_hi),
                            op=mybir.AluOpType.mult)
    u1 = small.tile([P, NT * PK * K], F32, tag="u1")
    u14 = u1[:].rearrange("p (t a b) -> p t a b", t=NT, b=K)
    in_lo = pt4[:, :, :, 0:K]
    nc.vector.tensor_tensor(u14, in_lo, wb(0, in_lo),
                            op=mybir.AluOpType.mult)
    cm = small.tile([P, NT * PK * K], F32, tag="cm")
    nc.vector.tensor_tensor(cm[:], t1[:], u1[:], op=mybir.AluOpType.add)
    cm4 = cm[:].rearrange("p (t a b) -> p t a b", t=NT, b=K)

    t2 = small.tile([P, NT * K * K], F32, tag="t2")
    t24 = t2[:].rearrange("p (t a b) -> p t a b", t=NT, b=K)
    cm_hi = cm4[:, :, 1:PK, :]
    nc.vector.tensor_tensor(t24, cm_hi, wb(3, cm_hi),
                            op=mybir.AluOpType.mult)
    u2 = small.tile([P, NT * K * K], F32, tag="u2")
    u24 = u2[:].rearrange("p (t a b) -> p t a b", t=NT, b=K)
    cm_lo = cm4[:, :, 0:K, :]
    nc.vector.tensor_tensor(u24, cm_lo, wb(2, cm_lo),
                            op=mybir.AluOpType.mult)
    otall = small.tile([P, NT * K * K], BF16, tag="otall")
    nc.vector.tensor_tensor(otall[:], t2[:], u2[:], op=mybir.AluOpType.add)

    nc.sync.dma_start(
        aps["out"].rearrange("(t p) k -> p t k", p=P),
        otall[:].rearrange("p (t k) -> p t k", k=K * K))


def build_program(geom, rep=1, unroll=1, ab=frozenset()):
    """rep = number of For_i iterations; each runs `unroll` bodies."""
    SROWS, BXF = geom
    nc = bacc.Bacc("TRN2", target_bir_lowering=False, debug=False,
                   num_devices=NCORES)
    aps = {}
    aps["f1s"] = nc.dram_tensor("f1s", [KH, P, QPC], BF16,
                                kind="ExternalInput").ap()
    aps["f2s"] = nc.dram_tensor("f2s", [NT, KH, P, BXF * SROWS], BF16,
                                kind="ExternalInput").ap()
    aps["meta"] = nc.dram_tensor("meta", [QPC, NM], I32,
                                 kind="ExternalInput").ap()
    aps["out"] = nc.dram_tensor("out", [QPC, K * K], BF16,
                                kind="ExternalOutput").ap()
    scr = [[nc.dram_tensor(f"scr{i}_{t}", [P * BXF * SROWS], BF16)
            for t in range(NT)] for i in range(min(2, unroll))]

    with tile.TileContext(nc) as tc:
        ctx = contextlib.ExitStack()
        with ctx:
            const = ctx.enter_context(tc.tile_pool(name="const", bufs=3))
            corr_pool = ctx.enter_context(tc.tile_pool(name="corr", bufs=3))
            psum_pool = ctx.enter_context(
                tc.tile_pool(name="ps", bufs=8, space="PSUM"))
            small = ctx.enter_context(tc.tile_pool(name="small", bufs=3))
            pools = (const, corr_pool, psum_pool, small)
            if rep == 1:
                for u in range(unroll):
                    _body(tc, nc, aps, scr[u % len(scr)], geom, pools, u, ab)
            else:
                with tc.For_i(0, rep):
                    for u in range(unroll):
                        _body(tc, nc, aps, scr[u % len(scr)], geom, pools,
                              u, ab)
    nc.compile()
    return nc


_PROGRAMS = {}


def kernel(fmap1, fmap2, coords, radius):
    assert int(radius) == R, f"kernel hardcodes radius=4, got {radius}"
    in_maps, order, geom = host_preprocess(fmap1, fmap2, coords)
    nc = _PROGRAMS.get(geom)
    if nc is None:
        nc = _PROGRAMS[geom] = build_program(geom)
    last_err = None
    for _ in range(3):  # the remote compile hook occasionally flakes
        try:
            res = bass_utils.run_bass_kernel_spmd(
                nc, in_maps, core_ids=list(range(NCORES)))
            return assemble_output(res.results, order)
        except Exception as e:  # noqa: BLE001
            last_err = e
    raise last_err
